# revision 3
# baseline (speedup 1.0000x reference)
"""Trainium2 Bass kernel for AdvancedKANLayer.

Math (per reference):
  xn = tanh(x)                                  (B, IN)
  d_g = |xn - g|                                for 8 grid points g
  f(d) = 2*(1-d)+^3 - 8*(0.5-d)+^3              (piecewise-cubic B-spline basis)
  out[b,o] = sum_{i,g} f(d_g[b,i]) * sw[o,i,g] + 0.1 * xn @ ba.T

Device formulation (per core, batch-sharded 8 ways, b_loc=512):
  mA = min(d-1, 0)   -> mA^3 = -(1-d)+^3
  mB = min(d-0.5, 0) -> mB^3 = -(0.5-d)+^3
  F  = 4*mB^3 - mA^3   and   f = 2*F
  out = W2.T @ [F channels (8 per i-tile), xn channel]   (single PE contraction,
        K = 4*(8+1)*128 = 4608), where W2 folds the *2 and the 0.1.

Layout: i on partitions (4 tiles of 128), b on free dim (512).
x is passed transposed per core: xT[i, b]. Output is [o, b] per core,
gathered + transposed on host.
"""

import sys

if "/opt/trn_rl_repo" not in sys.path:
    sys.path.insert(0, "/opt/trn_rl_repo")

import numpy as np

IN_F = 512
OUT_F = 512
GRID = 8
BATCH = 4096
NCORES = 8
B_LOC = BATCH // NCORES  # 512
NT = IN_F // 128         # 4 i-tiles
NO = OUT_F // 128        # 4 o-tiles
NCH = GRID + 1           # 8 basis channels + 1 xn channel per i-tile
NK = NT * NCH            # 36 k-tiles of 128

CFG = {
    "abs_on_act": 8,   # how many of the 8 |xn-g| ops go on ScalarE (rest on DVE)
    "sq_act_half": True,  # square of the A-chain on ScalarE, B-chain on DVE
}

_CACHE = {}


def _build(grid_vals, cfg):
    import concourse.tile as tile
    import concourse.mybir as mybir
    from concourse import bacc

    dt = mybir.dt
    f16 = dt.float16
    f32 = dt.float32
    AF = mybir.ActivationFunctionType
    OP = mybir.AluOpType

    nc = bacc.Bacc("TRN2", target_bir_lowering=False, debug=False)
    xT = nc.dram_tensor("xT", [IN_F, B_LOC], f32, kind="ExternalInput")
    w2 = nc.dram_tensor("w2", [NK * 128, OUT_F], f16, kind="ExternalInput")
    out = nc.dram_tensor("out", [OUT_F, B_LOC], f32, kind="ExternalOutput")

    with tile.TileContext(nc) as tc:
        with (
            tc.tile_pool(name="consts", bufs=1) as cpool,
            tc.tile_pool(name="w", bufs=1) as wpool,
            tc.tile_pool(name="x", bufs=2) as xpool,
            tc.tile_pool(name="elem", bufs=2) as epool,
            tc.tile_pool(name="fch", bufs=2) as fpool,
            tc.tile_pool(name="osb", bufs=2) as opool,
            tc.tile_pool(name="ps", bufs=1, space="PSUM") as pspool,
        ):
            # Per-partition bias constants -g for the ACT Abs ops.
            gbias = cpool.tile([128, GRID], f32)
            for g in range(GRID):
                nc.vector.memset(gbias[:, g : g + 1], -float(grid_vals[g]))

            # Extended weights, one SBUF tile [128, NK*OUT_F] (f16, 36KB/part).
            wbig = wpool.tile([128, NK, OUT_F], f16)
            w2ap = w2.ap().rearrange("(n p) o -> n p o", p=128)
            for kt in range(NK):
                nc.sync.dma_start(out=wbig[:, kt, :], in_=w2ap[kt])

            psums = [
                pspool.tile([128, B_LOC], f32, tag=f"ps{ot}", name=f"ps{ot}")
                for ot in range(NO)
            ]

            xTap = xT.ap().rearrange("(t p) b -> t p b", p=128)
            for t in range(NT):
                xt32 = xpool.tile([128, B_LOC], f32, tag="xt32")
                nc.sync.dma_start(out=xt32[:], in_=xTap[t])
                xn = xpool.tile([128, B_LOC], f16, tag="xn")
                nc.scalar.activation(xn[:], xt32[:], AF.Tanh)

                D = epool.tile([128, GRID * B_LOC], f16, tag="D")
                for g in range(GRID):
                    dst = D[:, g * B_LOC : (g + 1) * B_LOC]
                    if g < cfg["abs_on_act"]:
                        nc.scalar.activation(
                            dst, xn[:], AF.Abs, bias=gbias[:, g : g + 1], scale=1.0
                        )
                    else:
                        nc.vector.tensor_scalar(
                            dst, xn[:], float(grid_vals[g]), 0.0,
                            OP.subtract, OP.abs_max,
                        )

                GB = GRID * B_LOC  # 4096
                M = epool.tile([128, 2 * GB], f16, tag="M")
                nc.vector.tensor_scalar(M[:, :GB], D[:], 1.0, 0.0, OP.subtract, OP.min)
                nc.vector.tensor_scalar(M[:, GB:], D[:], 0.5, 0.0, OP.subtract, OP.min)

                SQ = epool.tile([128, 2 * GB], f16, tag="SQ")
                if cfg["sq_act_half"]:
                    nc.scalar.activation(SQ[:, :GB], M[:, :GB], AF.Square)
                    nc.vector.tensor_tensor(SQ[:, GB:], M[:, GB:], M[:, GB:], OP.mult)
                else:
                    nc.vector.tensor_tensor(SQ[:], M[:], M[:], OP.mult)
                CU = epool.tile([128, 2 * GB], f16, tag="CU")
                nc.vector.tensor_tensor(CU[:], SQ[:], M[:], OP.mult)

                # F = 4*mB^3 - mA^3  (f = 2F; the 2 is folded into W2)
                F = fpool.tile([128, GB], f16, tag="F")
                nc.vector.scalar_tensor_tensor(
                    F[:], CU[:, GB:], 4.0, CU[:, :GB], OP.mult, OP.subtract
                )

                for ot in range(NO):
                    for ch in range(NCH):
                        if ch < GRID:
                            rhs = F[:, ch * B_LOC : (ch + 1) * B_LOC]
                        else:
                            rhs = xn[:]
                        kt = t * NCH + ch
                        lhsT = wbig[:, kt, ot * 128 : (ot + 1) * 128]
                        nc.tensor.matmul(
                            psums[ot][:],
                            lhsT,
                            rhs,
                            start=(t == 0 and ch == 0),
                            stop=(t == NT - 1 and ch == NCH - 1),
                        )

            for ot in range(NO):
                osb = opool.tile([128, B_LOC], f32, tag="osb")
                nc.vector.tensor_copy(osb[:], psums[ot][:])
                nc.sync.dma_start(
                    out=out.ap()[ot * 128 : (ot + 1) * 128, :], in_=osb[:]
                )

    nc.compile()
    return nc


def _get_nc(grid_vals, cfg=None):
    cfg = cfg or CFG
    key = (tuple(np.asarray(grid_vals, np.float32).tolist()), tuple(sorted(cfg.items())))
    if key not in _CACHE:
        _CACHE[key] = _build(grid_vals, cfg)
    return _CACHE[key]


def _prep_inputs(x, spline_weight, base_activation):
    x = np.asarray(x, np.float32)
    sw = np.asarray(spline_weight, np.float32)
    ba = np.asarray(base_activation, np.float32)
    # W2[k, o] with k = (t*NCH + ch)*128 + p ; ch<8 -> 2*sw[o, i, g] ; ch==8 -> 0.1*ba[o, i]
    W2 = np.empty((NK, 128, OUT_F), np.float32)
    sw_t = sw.transpose(1, 2, 0)  # [in, g, out]
    ba_t = ba.T  # [in, out]
    for t in range(NT):
        isl = slice(t * 128, (t + 1) * 128)
        for g in range(GRID):
            W2[t * NCH + g] = 2.0 * sw_t[isl, g, :]
        W2[t * NCH + GRID] = 0.1 * ba_t[isl, :]
    W2 = W2.reshape(NK * 128, OUT_F).astype(np.float16)
    xT = np.ascontiguousarray(x.T)  # [IN_F, BATCH]
    in_maps = [
        {
            "xT": np.ascontiguousarray(xT[:, c * B_LOC : (c + 1) * B_LOC]),
            "w2": W2,
        }
        for c in range(NCORES)
    ]
    return in_maps


def _run(x, spline_weight, base_activation, grid_points, trace=False, cfg=None,
         tmpdir=None):
    from concourse.bass_utils import run_bass_kernel_spmd

    nc = _get_nc(np.asarray(grid_points, np.float32), cfg)
    in_maps = _prep_inputs(x, spline_weight, base_activation)
    res = run_bass_kernel_spmd(
        nc, in_maps, list(range(NCORES)), trace=trace, tmpdir=tmpdir
    )
    outs = [res.results[c]["out"] for c in range(NCORES)]  # each [OUT_F, B_LOC]
    full = np.concatenate(outs, axis=1)  # [OUT_F, BATCH]
    return np.ascontiguousarray(full.T.astype(np.float32)), res


def kernel(x, spline_weight, base_activation, grid_points):
    out, _ = _run(x, spline_weight, base_activation, grid_points)
    return out
